# revision 43
# baseline (speedup 1.0000x reference)
"""Trainium2 Bass kernel for the non-local-attention block (nn_DNL_74234214744693).

Reference (B=4, C=64, H=W=64, N=4096):
    k = conv1x1(x,kw,kb); k_wh = k - mean_j(k)
    q = conv1x1(x,qw,qb)                      (q-whitening is a softmax no-op)
    qk[b,i,j] = sum_c k_wh[b,c,i] q[b,c,j]
    m  = conv1x1(x,mw,mb) -> [B,N];  mm[b,i,j] = m[b,i]*m[b,j]
    f  = softmax_j(qk) + softmax_b(mm)        (second softmax over BATCH)
    y  = einsum('bci,bij->bcj', v, f) + BN(conv1x1(x,ww,wb))

Key tricks (v3):
  * Associativity: qk = k_wh^T (qT^T x) = (qT @ k_wh)^T x.  The per-sample
    [65,512] matrix kq = qT @ k_wh becomes the qk stationary; the moving
    operand is x itself (fp8, resident).  The whole q conv and its PSUM->SBUF
    copies vanish.
  * e2 = exp(m_i m_j) has a rank-1 argument -> exp replaced by a degree-12
    polynomial evaluated as PE matmuls over a stacked (b,k) power basis
    (stored as (m/2)^k, fp8 hi/lo split, DoubleRow).  D = sum_b e2_b and
    h_b = e2_b - D/4 come from the same basis via row weights.
  * y2 mean-subtraction: y2 = S/4 + (v/4) @ g', g' = 4*f2 - 1 (fp8-safe).
    S = sum_n v via ones-matmul; added in the out-copy (Identity+bias).
  * fp8e4m3 + DoubleRow for qk / poly-h/D / y1 / g-y / wx (zero-padded
    stationaries; stride-0 second subtile plane for the moving operands).
  * softmax_j via constant shift exp(qk-7); Z free from accum_out; v1p=16v/Z.
    PSUM accumulates x16; out-copy applies /16 and adds S/4.
  * Decoupled PSUM pools: psQ (qk+exp rotation) never waits on the slow
    elementwise consumers that drain psHY (poly/g/y/convs).

Sharding: each of 8 cores owns a 512-row i-slice of the [N,N] maps for all 4
samples; host sums the 8 partial outputs (wx is pre-divided by 8).
"""

import functools

import numpy as np
import ml_dtypes

N_CORES = 8
B, C, H, W = 4, 64, 64, 64
N = H * W                 # 4096
SL = N // N_CORES         # 512
NIT = SL // 128           # 4
NITP = NIT // 2           # 2
NJ5 = N // 512            # 8
NJQ = N // 1024           # 4
EPS = 1e-5
SHIFT = 7.0
DEG = 12
KP = 16

# exp(t) ~= sum_k POLY_A[k] t^k on [-4, 4]; max abs err 4e-6.
POLY_A = [1.000000481756752, 0.9999888881522239, 0.49999706307401615,
          0.16668597667298232, 0.04166958451576583, 0.008323772405684203,
          0.0013878046435380107, 0.00020043162670676482,
          2.4992571180864735e-05, 2.550512749531329e-06,
          2.5846139980280564e-07, 3.4818470661121456e-08,
          2.8304950257085147e-09]

F8 = ml_dtypes.float8_e4m3
BF16 = ml_dtypes.bfloat16


def _build_program():
    import concourse.bass as bass
    import concourse.tile as tile
    from concourse import bacc, masks, mybir

    dt = mybir.dt
    AF = mybir.ActivationFunctionType
    ALU = mybir.AluOpType
    DR = mybir.MatmulPerfMode.DoubleRow

    nc = bacc.Bacc("TRN2", target_bir_lowering=False, debug=False,
                   enable_asserts=False, num_devices=1)

    # ---------------- DRAM I/O ----------------
    x_ext = nc.dram_tensor("x_ext", [B, C + 1, N], dt.bfloat16, kind="ExternalInput")
    xsl_ext = nc.dram_tensor("xsl_ext", [B, C + 1, SL], dt.bfloat16, kind="ExternalInput")
    x_f8 = nc.dram_tensor("x_f8", [B, 128, N], dt.float8e4, kind="ExternalInput")
    qTT = nc.dram_tensor("qTT", [C, C + 1], dt.bfloat16, kind="ExternalInput")
    kT = nc.dram_tensor("kT", [C + 1, C], dt.bfloat16, kind="ExternalInput")
    vmT = nc.dram_tensor("vmT", [C + 1, C + 1], dt.bfloat16, kind="ExternalInput")
    mwT = nc.dram_tensor("mwT", [C + 1, 1], dt.bfloat16, kind="ExternalInput")
    wT16 = nc.dram_tensor("wT16", [128, 2, 2, 2 * C], dt.float8e4, kind="ExternalInput")
    au_in = nc.dram_tensor("au_in", [64, 8], dt.float32, kind="ExternalInput")
    zeros8 = nc.dram_tensor("zeros8", [64, 8192], dt.float8e4, kind="ExternalInput")
    y_part = nc.dram_tensor("y_part", [B, C, N], dt.float32, kind="ExternalOutput")

    f8_, bf_, f32 = dt.float8e4, dt.bfloat16, dt.float32

    with tile.TileContext(nc) as tc:
        from contextlib import ExitStack

        with ExitStack() as top:
            consts = top.enter_context(tc.tile_pool(name="consts", bufs=1))
            p_xf8 = top.enter_context(tc.tile_pool(name="p_xf8", bufs=B))
            p_kq = top.enter_context(tc.tile_pool(name="p_kq", bufs=1))
            p_f1 = top.enter_context(tc.tile_pool(name="p_f1", bufs=B * NITP))
            p_vT = top.enter_context(tc.tile_pool(name="p_vT", bufs=B * NIT))
            p_vp = top.enter_context(tc.tile_pool(name="p_vp", bufs=2))
            p_us = top.enter_context(tc.tile_pool(name="p_us", bufs=1))
            p_vdr = top.enter_context(tc.tile_pool(name="p_vdr", bufs=1))
            p_tit = top.enter_context(tc.tile_pool(name="p_tit", bufs=NIT))
            p_sm = top.enter_context(tc.tile_pool(name="p_sm", bufs=48))
            p_zp = top.enter_context(tc.tile_pool(name="p_zp", bufs=24))
            p_out = top.enter_context(tc.tile_pool(name="p_out", bufs=2))
            psQ = top.enter_context(tc.tile_pool(name="psQ", bufs=2, space="PSUM"))
            psH = top.enter_context(tc.tile_pool(name="psH", bufs=2, space="PSUM"))

            # ---------------- consts ----------------
            sb_qTT = consts.tile([C, C + 1], bf_)
            sb_kT = consts.tile([C + 1, C], bf_)
            sb_vmT = consts.tile([C + 1, C + 1], bf_)
            sb_mwT = consts.tile([C + 1, 1], bf_)
            sb_wT = consts.tile([128, 2, 2, 2 * C], f8_)
            sb_au = consts.tile([64, 8], f32)
            ident = consts.tile([128, 128], f32)
            bias_t = consts.tile([128, 1], f32)
            ones_t = consts.tile([128, 1], bf_)
            nc.scalar.dma_start(sb_qTT, qTT.ap())
            nc.scalar.dma_start(sb_kT, kT.ap())
            nc.scalar.dma_start(sb_vmT, vmT.ap())
            nc.scalar.dma_start(sb_mwT, mwT.ap())
            nc.scalar.dma_start(sb_wT, wT16.ap())
            nc.scalar.dma_start(sb_au, au_in.ap())
            masks.make_identity(nc, ident[:])
            nc.vector.memset(bias_t, -SHIFT)
            nc.vector.memset(ones_t, 1.0)

            # ---------------- persistent tiles ----------------
            xf8 = [p_xf8.tile([128, N], f8_, name=f"xf8_{b}", tag="xf8") for b in range(B)]
            kq_dr = p_kq.tile([128, 2, B * SL], f8_)
            f1 = [[p_f1.tile([128, 2, N], f8_, name=f"f1_{b}_{p}", tag="f1")
                   for p in range(NITP)] for b in range(B)]
            v_T = [[p_vT.tile([128, C], bf_, name=f"v_T{b}_{i}", tag="v_T")
                    for i in range(NIT)] for b in range(B)]
            v1p_all = p_vp.tile([128, 2, B * NITP * 2 * C], f8_)
            vT4_all = p_vp.tile([128, 2, B * NITP * 2 * C], f8_)
            u_all = p_us.tile([128, 2, (B + 1) * SL], f8_)
            u_st = [u_all[:, :, x * SL:(x + 1) * SL] for x in range(B + 1)]
            v_dr = p_vdr.tile([128, 2, N], f8_)
            t_it = [p_tit.tile([128, B, KP], f32, name=f"t_it{i}", tag="t_it")
                    for i in range(NIT)]
            sq = [p_sm.tile([128, 1], f32, name=f"sq{p}", tag="sq") for p in range(2)]

            # ---------------- zero-fill DR pads ----------------
            z = zeros8.ap()
            nc.sync.dma_start(v_dr[64:128, 0, :], z[:, 0:N])
            nc.sync.dma_start(v_dr[64:128, 1, :], z[:, 0:N])
            nc.sync.dma_start(kq_dr[65:128, 0, :], z[0:63, 0:B * SL])
            nc.sync.dma_start(kq_dr[0:64, 1, :], z[:, 0:B * SL])
            nc.sync.dma_start(kq_dr[64:128, 1, :], z[:, 0:B * SL])
            for t2 in (v1p_all, vT4_all):
                nc.sync.dma_start(t2[0:64, 0, :], z[:, 0:B * NITP * 2 * C])
                nc.sync.dma_start(t2[0:64, 1, :], z[:, 0:B * NITP * 2 * C])
                nc.sync.dma_start(t2[64:128, 0, :], z[:, 0:B * NITP * 2 * C])
                nc.sync.dma_start(t2[64:128, 1, :], z[:, 0:B * NITP * 2 * C])
            nc.sync.dma_start(u_all[64:128, 0, :], z[:, 0:(B + 1) * SL])
            nc.sync.dma_start(u_all[64:128, 1, :], z[:, 0:(B + 1) * SL])

            def vst(all_t, bb, itp):
                o = (bb * NITP + itp) * 2 * C
                return all_t[:, :, o:o + 2 * C]

            def vwr(all_t, bb, itp, s_):
                o = (bb * NITP + itp) * 2 * C + (bb % 2) * C
                return all_t[:, s_, o:o + C]

            def dr_mov(tile2d, jsl):
                return tile2d[:, jsl].unsqueeze(1).to_broadcast(
                    [128, 2, jsl.stop - jsl.start])

            hd_q = [(jq, rit) for jq in range(NJQ) for rit in range(NIT)]
            HD_PACE = [0, 0, 0, 2, 2, 2, 2, 2, 1, 1, 1, 1, 1, 1, 0, 0]
            st = {"slot": 0, "y_done": 0}

            def emit_slot(b, it):
                itp, s_ = it // 2, it % 2
                slot = st["slot"]
                rounds_done = 16 - len(hd_q)
                if slot >= 8 and st["y_done"] < 2 * (rounds_done // 4) \
                        and st["y_done"] < NJ5:
                    y_round(0, st["y_done"])
                    st["y_done"] += 1
                for _ in range(HD_PACE[slot]):
                    if hd_q:
                        hd_round(*hd_q.pop(0))
                zp = [p_zp.tile([128, 1], f32, name=f"zp{_h}", tag="zp")
                      for _h in range(4)]
                for quar in range(4):
                    psk = psQ.tile([128, 1024], f32, name=f"qk{b}_{it}_{quar}",
                                   tag="psQ")
                    for q2 in range(2):
                        j0 = quar * 1024 + q2 * 512
                        nc.tensor.matmul(
                            psk[:, q2 * 512:(q2 + 1) * 512],
                            kq_dr[:, :, b * SL + it * 128:b * SL + (it + 1) * 128],
                            dr_mov(xf8[b], slice(j0, j0 + 512)),
                            start=True, stop=True, perf_mode=DR)
                    nc.scalar.activation(
                        f1[b][itp][:, s_, quar * 1024:(quar + 1) * 1024],
                        psk, AF.Exp, bias=bias_t[:], accum_out=zp[quar])
                z1a = p_sm.tile([128, 1], f32, name="z1a", tag="z1a")
                z1b = p_sm.tile([128, 1], f32, name="z1b", tag="z1b")
                z1 = p_sm.tile([128, 1], f32, name="z1", tag="z1")
                rz = p_sm.tile([128, 1], f32, name="rz", tag="rz")
                nc.gpsimd.tensor_tensor(z1a, zp[0], zp[1], op=ALU.add)
                nc.gpsimd.tensor_tensor(z1b, zp[2], zp[3], op=ALU.add)
                nc.gpsimd.tensor_tensor(z1, z1a, z1b, op=ALU.add)
                nc.vector.reciprocal_approx_fast(rz, z1)
                nc.gpsimd.tensor_scalar(vwr(v1p_all, b, itp, s_), v_T[b][it],
                                        scalar1=rz, scalar2=16.0,
                                        op0=ALU.mult, op1=ALU.mult)
                st["slot"] += 1

            with ExitStack() as p01:
                p_vlad = p01.enter_context(tc.tile_pool(name="p_vlad", bufs=1))
                p_x = p01.enter_context(tc.tile_pool(name="p_x", bufs=2))
                p_xsl = p01.enter_context(tc.tile_pool(name="p_xsl", bufs=2))
                p_kwh = p01.enter_context(tc.tile_pool(name="p_kwh", bufs=2))
                v_lad = p_vlad.tile([128, 32, B, KP], f32)

                # ---------------- P0: per-sample convs ----------------
                for b in range(B):
                    x_sb = p_x.tile([C + 1, N], bf_, name=f"x_sb{b}", tag="x_sb")
                    xsl_sb = p_xsl.tile([C + 1, SL], bf_, name=f"xsl{b}", tag="xsl")
                    nc.sync.dma_start(x_sb, x_ext.ap()[b])
                    nc.sync.dma_start(xsl_sb, xsl_ext.ap()[b])
                    nc.sync.dma_start(xf8[b], x_f8.ap()[b])

                    # xu = mean_j x via DVE 2x tensor_scalar with accum
                    # (throwaway bf16 output into not-yet-written f1 space)
                    xu_f = p_sm.tile([C + 1, 1], f32, name=f"xu_f{b}", tag="xu_f")
                    xu_bf = p_sm.tile([C + 1, 1], bf_, name=f"xu_bf{b}", tag="xu_bf")
                    xuh = [p_sm.tile([C + 1, 1], f32, name=f"xuh{_h}", tag="xuh")
                           for _h in range(2)]
                    xud = f1[3][1][:, :, :].bitcast(bf_)
                    xeng = nc.vector
                    for _h in range(2):
                        xeng.tensor_scalar(xud[0:C + 1, _h, :],
                                           x_sb[:, _h * 2048:(_h + 1) * 2048],
                                           1.0 / N, 0.0, op0=ALU.mult,
                                           op1=ALU.add, accum_out=xuh[_h])
                    xeng.tensor_tensor(xu_f, xuh[0], xuh[1], op=ALU.add)
                    xeng.tensor_copy(xu_bf, xu_f)

                    # vm/mT/ku/S gen (psH), k conv in a second gen
                    ps = psH.tile([128, 512], f32, name=f"vm{b}", tag="psH")
                    for it in range(NIT):
                        nc.tensor.matmul(ps[:, it * 65:(it + 1) * 65],
                                         xsl_sb[:, it * 128:(it + 1) * 128],
                                         sb_vmT, start=True, stop=True)
                    for t in range(32):
                        nc.tensor.matmul(ps[:, 264 + t:265 + t],
                                         x_sb[:, t * 128:(t + 1) * 128],
                                         sb_mwT, start=True, stop=True)
                    nc.tensor.matmul(ps[0:C, 300:301], sb_kT, xu_bf,
                                     start=True, stop=True)
                    psk2 = psH.tile([128, 512], f32, name=f"kc{b}", tag="psH")
                    nc.tensor.matmul(psk2[0:C, :], sb_kT, xsl_sb,
                                     start=True, stop=True)

                    for it in range(NIT):
                        cs = slice(it * 65, it * 65 + C)
                        nc.vector.tensor_copy(v_T[b][it], ps[:, cs])
                        nc.gpsimd.tensor_scalar_mul(vwr(vT4_all, b, it // 2, it % 2),
                                                    v_T[b][it], 4.0)
                        nc.vector.tensor_scalar_mul(t_it[it][:, b, 1:2],
                                                    ps[:, it * 65 + C:it * 65 + C + 1], 0.5)
                    nc.vector.tensor_scalar_mul(v_lad[:, :, b, 1], ps[:, 264:296], 0.5)

                    negku = p_sm.tile([C, 1], f32, name=f"negku{b}", tag="negku")
                    nc.vector.tensor_scalar_mul(negku, ps[0:C, 300:301], -1.0)
                    kwh_sb = p_kwh.tile([C, SL], bf_, name=f"kwh{b}", tag="kwh")
                    nc.scalar.activation(kwh_sb, psk2[0:C, :], AF.Identity,
                                         bias=negku[:], scale=1.0)

                    # S = sum_n v (ones matmul, accumulated over it)
                    po = (b % 2) * 64
                    for it in range(NIT):
                        nc.tensor.matmul(ps[po:po + C, 301:302], v_T[b][it], ones_t,
                                         start=(it == 0), stop=(it == NIT - 1))
                    nc.vector.tensor_scalar_mul(sq[b // 2][po:po + C, :],
                                                ps[po:po + C, 301:302], 0.25)

                    # kq = qT @ k_wh  -> fp8 DR stationary for the qk matmuls
                    ps2 = psH.tile([128, 512], f32, name=f"kq{b}", tag="psH")
                    nc.tensor.matmul(ps2[0:C + 1, 0:SL], sb_qTT, kwh_sb,
                                     start=True, stop=True)
                    nc.scalar.copy(kq_dr[0:C + 1, 0, b * SL:(b + 1) * SL],
                                   ps2[0:C + 1, 0:SL])

                for _it in range(3):
                    emit_slot(0, _it)

                # ---------------- P1: power ladders + transposes ----------------
                nc.gpsimd.memset(v_lad[:, :, :, 0:1], 1.0)
                for it in range(NIT):
                    nc.gpsimd.memset(t_it[it][:, :, 0:1], 1.0)
                for k in range(2, DEG + 1):
                    nc.gpsimd.tensor_tensor(v_lad[:, :, :, k], v_lad[:, :, :, k - 1],
                                            v_lad[:, :, :, 1], op=ALU.mult)
                for it in range(NIT):
                    for k in range(2, DEG + 1):
                        nc.gpsimd.tensor_tensor(t_it[it][:, :, k], t_it[it][:, :, k - 1],
                                                t_it[it][:, :, 1], op=ALU.mult)

                # V transposes: [128 j, (b,k)] -> [(b,k), j] with fp8 hi/lo
                for gen in range(8):
                    psv = psH.tile([128, 512], f32, name=f"vt{gen}", tag="psH")
                    for j4 in range(4):
                        jt = gen * 4 + j4
                        nc.tensor.transpose(psv[0:64, j4 * 128:(j4 + 1) * 128],
                                            v_lad[:, jt], ident)
                    csl = slice(gen * 512, (gen + 1) * 512)
                    nc.vector.tensor_copy(v_dr[0:64, 0, csl], psv[0:64, :])
                    nc.vector.tensor_tensor(v_dr[0:64, 1, csl], psv[0:64, :],
                                            v_dr[0:64, 0, csl], op=ALU.subtract)

                # U transposes + stationaries
                psu = psH.tile([128, 512], f32, name="ut", tag="psH")
                for it in range(NIT):
                    nc.tensor.transpose(psu[0:64, it * 128:(it + 1) * 128],
                                        t_it[it], ident)
                for x in range(B + 1):
                    auc = sb_au[:, x:x + 1]
                    nc.vector.tensor_scalar_mul(u_st[x][0:64, 0, :],
                                                psu[0:64, 0:SL], auc[0:64])
                    nc.vector.scalar_tensor_tensor(u_st[x][0:64, 1, :], psu[0:64, 0:SL],
                                                   auc[0:64], u_st[x][0:64, 0, :],
                                                   op0=ALU.mult, op1=ALU.subtract)

            # ---------------- P2: qk/exp + poly/g + y ----------------
            # (pools opened after P0/P1 scope closed -> reuse v_lad space)
            p_g0 = top.enter_context(tc.tile_pool(name="p_g0", bufs=16))
            p_g1 = top.enter_context(tc.tile_pool(name="p_g1", bufs=16))
            p_rr = top.enter_context(tc.tile_pool(name="p_rr", bufs=2))
            g_tiles = {}
            out_tiles = {}

            def hd_round(jq, rit):
                itp, s = rit // 2, rit % 2
                i_sl = slice(rit * 128, (rit + 1) * 128)
                rr = p_rr.tile([128, 1024], f32, name=f"rr{jq}_{rit}", tag="rr")
                psd = psH.tile([128, 1024], f32, name=f"hdD{jq}_{rit}", tag="psH")
                for hh in range(2):
                    jsl = slice(jq * 1024 + hh * 512, jq * 1024 + (hh + 1) * 512)
                    nc.tensor.matmul(psd[:, hh * 512:(hh + 1) * 512],
                                     u_st[B][:, :, i_sl], v_dr[:, :, jsl],
                                     start=True, stop=True, perf_mode=DR)
                nc.vector.reciprocal_approx_fast(rr, psd)
                for bb in range(B):
                    key = (bb, itp, jq)
                    if key not in g_tiles:
                        pool = p_g0 if bb < 2 else p_g1
                        g_tiles[key] = pool.tile([128, 2, 1024], f8_,
                                                 name=f"g{bb}_{itp}_{jq}", tag="g")
                    psh_ = psH.tile([128, 1024], f32,
                                    name=f"hd{bb}_{jq}_{rit}", tag="psH")
                    for hh in range(2):
                        jsl = slice(jq * 1024 + hh * 512, jq * 1024 + (hh + 1) * 512)
                        nc.tensor.matmul(psh_[:, hh * 512:(hh + 1) * 512],
                                         u_st[bb][:, :, i_sl], v_dr[:, :, jsl],
                                         start=True, stop=True, perf_mode=DR)
                    nc.vector.tensor_tensor(g_tiles[key][:, s, :], psh_, rr,
                                            op=ALU.mult)

            def y_round(pair, j5, tail=False):
                jsl = slice(j5 * 512, (j5 + 1) * 512)
                jq, jh = j5 // 2, j5 % 2
                if tail:
                    ps = psQ.tile([128, 1024], f32, name=f"y{pair}_{j5}", tag="psQ")
                else:
                    ps = psH.tile([128, 512], f32, name=f"y{pair}_{j5}", tag="psH")
                reg = ps[:, 0:512]
                for i in range(2):
                    bb = 2 * pair + i
                    nc.tensor.matmul(reg, sb_wT[:, :, i, :], dr_mov(xf8[bb], jsl),
                                     start=(i == 0), stop=False, perf_mode=DR)
                    for itp in range(NITP):
                        nc.tensor.matmul(reg, vst(v1p_all, bb, itp),
                                         f1[bb][itp][:, :, jsl],
                                         start=False, stop=False, perf_mode=DR)
                        nc.tensor.matmul(
                            reg, vst(vT4_all, bb, itp),
                            g_tiles[(bb, itp, jq)][:, :, jh * 512:(jh + 1) * 512],
                            start=False,
                            stop=(i == 1 and itp == NITP - 1), perf_mode=DR)
                if (pair, jq) not in out_tiles:
                    out_tiles[(pair, jq)] = p_out.tile(
                        [128, 1024], f32, name=f"o{pair}_{jq}", tag="out_sb")
                out_sb = out_tiles[(pair, jq)]
                osl = slice(jh * 512, (jh + 1) * 512)
                nc.scalar.activation(out_sb[:, osl], ps[:, 0:512], AF.Identity,
                                     bias=sq[pair][:], scale=1.0 / 16.0)
                if jh == 1:
                    for i in range(2):
                        bb = 2 * pair + i
                        nc.sync.dma_start(y_part.ap()[bb][:, jq * 1024:(jq + 1) * 1024],
                                          out_sb[i * 64:i * 64 + C, :])

            for b in range(B):
                for it in range(NIT):
                    if b == 0 and it < 3:
                        continue
                    emit_slot(b, it)

            while hd_q:
                hd_round(*hd_q.pop(0))
            while st["y_done"] < NJ5:
                y_round(0, st["y_done"], tail=True)
                st["y_done"] += 1
            for j5 in range(NJ5):
                y_round(1, j5, tail=True)

    nc.compile()
    return nc


@functools.lru_cache(maxsize=1)
def _get_program():
    return _build_program()


def _prep_inputs(inputs):
    x = np.asarray(inputs["x"], np.float32).reshape(B, C, N)
    ones = np.ones((B, 1, N), np.float32)
    x65 = np.concatenate([x, ones], axis=1)                         # [B,65,N]
    x_ext = x65.astype(BF16)
    x_f8 = np.zeros((B, 128, N), F8)
    x_f8[:, :C + 1] = x65.astype(F8)

    qw = np.asarray(inputs["qw"], np.float32)
    qb = np.asarray(inputs["qb"], np.float32)
    kw = np.asarray(inputs["kw"], np.float32)
    kb = np.asarray(inputs["kb"], np.float32)
    mw = np.asarray(inputs["mw"], np.float32)
    mb = np.asarray(inputs["mb"], np.float32)
    vw = np.asarray(inputs["vw"], np.float32)
    vb = np.asarray(inputs["vb"], np.float32)
    ww = np.asarray(inputs["ww"], np.float32)
    wb = np.asarray(inputs["wb"], np.float32)
    g = np.asarray(inputs["bn_gamma"], np.float32)
    be = np.asarray(inputs["bn_beta"], np.float32)
    rm = np.asarray(inputs["bn_rm"], np.float32)
    rv = np.asarray(inputs["bn_rv"], np.float32)

    qTTa = np.concatenate([qw, qb[:, None]], axis=1)                # [64,65]
    kTa = np.concatenate([kw.T, kb[None, :]], axis=0)               # [65,64]

    vmT = np.zeros((C + 1, C + 1), np.float32)
    vmT[:C, :C] = vw.T
    vmT[C, :C] = vb
    vmT[:C, C] = mw[0]
    vmT[C, C] = mb[0]

    mwT = np.concatenate([mw[0][:, None], mb[:, None]], axis=0)     # [65,1]

    inv = g / np.sqrt(rv + EPS)
    wT_bn = np.zeros((C + 1, C), np.float32)
    wT_bn[:C, :] = (ww * inv[:, None]).T
    wT_bn[C, :] = wb * inv + be - rm * inv
    wT16 = np.zeros((128, 2, 2, 2 * C), np.float32)
    for i in range(2):
        wT16[0:C + 1, 0, i, i * C:(i + 1) * C] = (16.0 / N_CORES) * wT_bn

    # au rows ordered (b'*16 + k): weights for the (m/2)^k power basis.
    au = np.zeros((64, 8), np.float32)
    for bp in range(B):
        for k in range(DEG + 1):
            a4 = POLY_A[k] * (4.0 ** k)
            for bt in range(B):
                au[bp * KP + k, bt] = a4 * (0.75 if bp == bt else -0.25)
            au[bp * KP + k, 4] = a4 * 0.25
    zeros8_a = np.zeros((64, 8192), F8)

    common = {
        "x_ext": x_ext,
        "x_f8": x_f8,
        "qTT": qTTa.astype(BF16),
        "kT": kTa.astype(BF16),
        "vmT": vmT.astype(BF16),
        "mwT": mwT.astype(BF16),
        "wT16": wT16.astype(F8),
        "au_in": au,
        "zeros8": zeros8_a,
    }
    in_maps = []
    for ic in range(N_CORES):
        m = dict(common)
        m["xsl_ext"] = np.ascontiguousarray(x_ext[:, :, ic * SL:(ic + 1) * SL])
        in_maps.append(m)
    return in_maps


def kernel(**inputs):
    from concourse.bass_utils import run_bass_kernel_spmd

    nc = _get_program()
    in_maps = _prep_inputs(inputs)
    res = run_bass_kernel_spmd(nc, in_maps, core_ids=list(range(N_CORES)))
    y = np.zeros((B, C, N), np.float32)
    for r in res.results:
        y += r["y_part"]
    return y.reshape(B, C, H, W)


if __name__ == "__main__":
    rng = np.random.default_rng(0)
    ins = {
        "x": rng.standard_normal((B, C, H, W), dtype=np.float32),
        "qw": rng.standard_normal((C, C), dtype=np.float32) * 0.05,
        "qb": rng.standard_normal((C,), dtype=np.float32) * 0.05,
        "kw": rng.standard_normal((C, C), dtype=np.float32) * 0.05,
        "kb": rng.standard_normal((C,), dtype=np.float32) * 0.05,
        "mw": rng.standard_normal((1, C), dtype=np.float32) * 0.05,
        "mb": rng.standard_normal((1,), dtype=np.float32) * 0.05,
        "vw": rng.standard_normal((C, C), dtype=np.float32) * 0.05,
        "vb": rng.standard_normal((C,), dtype=np.float32) * 0.05,
        "ww": rng.standard_normal((C, C), dtype=np.float32) * 0.05,
        "wb": rng.standard_normal((C,), dtype=np.float32) * 0.05,
        "bn_gamma": np.ones((C,), np.float32),
        "bn_beta": np.zeros((C,), np.float32),
        "bn_rm": np.zeros((C,), np.float32),
        "bn_rv": np.ones((C,), np.float32),
    }
    out = kernel(**ins)
    print("kernel output", out.shape, out.dtype, np.abs(out).mean())
